# revision 12
# baseline (speedup 1.0000x reference)
"""Trainium2 Bass kernel for nn_Amplituedro (weighted embedding lookup).

path[b] = (sum_k w[b,k] * vertices[idx[b,k]]) / sum_k w[b,k]
eff     = mean_b ||path[b]||

Strategy (data-parallel over batch, 8 cores, 8192 rows/core):
  per 256-row pair of 128-row tiles:
    - DVE: row totals + reciprocals; cast w to bf16; scatter offsets k*64+idx
    - GPSIMD local_scatter: one-hot weight rows eqw[b, t*512+k*64+e] = w_bf16
    - DVE add tree: reduce k -> agg[b, (t,e)]  [128, 128] bf16
    - DMA xbar transpose: aggT[(t,e), b]
    - PE: path_psum[128,512] = aggT_t.T @ V_bf16 ; m1 = aggT_t.T @ G (G = V V^T)
    - DVE tensor_tensor_reduce: sqnorm_raw[b] = sum_e m1[b,e]*agg[b,e]
    - ACT: copy PSUM->SBUF bf16 with per-row scale = 1/total  (normalization)
    - DMA out bf16
  epilogue: norm = sqrt(sqnorm_raw)*recip, per-partition partial sums -> host.
Host: concat path shards (bf16->f32), sum efficiency partials / B.
"""

import numpy as np

N_CORES = 8
B_FULL = 65536
B_CORE = B_FULL // N_CORES  # 8192
K = 8
E = 64
D = 512
NT = B_CORE // 128  # 64 tiles per core
NPAIR = NT // 2  # 32

_CACHE = {}


def _build():
    import concourse.bass as bass
    import concourse.bacc as bacc
    import concourse.mybir as mybir
    import concourse.tile as tile

    f32 = mybir.dt.float32
    bf16 = mybir.dt.bfloat16
    i32 = mybir.dt.int32
    i16 = mybir.dt.int16
    ALU = mybir.AluOpType
    AF = mybir.ActivationFunctionType

    nc = bacc.Bacc(None, target_bir_lowering=False, debug=False)

    idx_d = nc.declare_dram_parameter("expert_indices", [B_CORE, K], i32, isOutput=False)
    w_d = nc.declare_dram_parameter("expert_weights", [B_CORE, K], f32, isOutput=False)
    # vertices pre-cast to bf16 on host; vertices_bf is V stacked twice
    # ([2, 64, 512]) so both PE partition halves hold a copy.
    v_d = nc.declare_dram_parameter("vertices_bf", [2, E, D], bf16, isOutput=False)
    vt_d = nc.declare_dram_parameter("vertices_t_bf", [D, E], bf16, isOutput=False)
    path_d = nc.declare_dram_parameter("path", [B_CORE, D], bf16, isOutput=True)
    eff_d = nc.declare_dram_parameter("eff", [128, 1], f32, isOutput=True)

    with tile.TileContext(nc) as tc:
        with (
            tc.tile_pool(name="const", bufs=1) as cpool,
            tc.tile_pool(name="work", bufs=3) as pool,
            tc.tile_pool(name="out", bufs=4) as opool,
            tc.tile_pool(name="ppath", bufs=2, space="PSUM") as ppool,
            tc.tile_pool(name="psmall", bufs=2, space="PSUM") as pspool,
        ):
            # ---- constants / V prep ----
            # V replicated into both partition halves so the t=1 matmul
            # (lhsT at base partition 64) has a matching-base rhs.
            vb = cpool.tile([128, D], bf16)
            nc.sync.dma_start(vb[:], v_d.rearrange("two e d -> (two e) d"))

            # VT [512, 64] -> [128 part, 4 chunk, 64]
            vtb = cpool.tile([128, 4, E], bf16)
            nc.sync.dma_start(vtb[:], vt_d.rearrange("(c p) e -> p c e", p=128))

            # G = V @ V.T  [64, 64]
            g_ps = pspool.tile([E, E], f32)
            for c in range(4):
                nc.tensor.matmul(
                    g_ps[:], vtb[:, c, :], vtb[:, c, :], start=(c == 0), stop=(c == 3)
                )
            gb = cpool.tile([128, E], bf16)
            nc.vector.tensor_copy(gb[0:64, :], g_ps[:])
            nc.vector.tensor_copy(gb[64:128, :], g_ps[:])

            # scatter offset bias: t*512 + k*64
            kvec = cpool.tile([128, 2, K], i32)
            nc.gpsimd.iota(kvec[:], pattern=[[512, 2], [64, K]], base=0, channel_multiplier=0)

            recips = cpool.tile([128, NT], f32)
            sqnorms = cpool.tile([128, NT], f32)

            for j in range(NPAIR):
                r0 = j * 256
                idx2 = pool.tile([128, 2, K], i32)
                nc.sync.dma_start(idx2[:], idx_d[r0 : r0 + 256, :].rearrange("(t p) k -> p t k", t=2))
                w2 = pool.tile([128, 2, K], f32)
                nc.sync.dma_start(w2[:], w_d[r0 : r0 + 256, :].rearrange("(t p) k -> p t k", t=2))

                tot2 = pool.tile([128, 2], f32)
                nc.vector.tensor_reduce(tot2[:], w2[:], axis=mybir.AxisListType.X, op=ALU.add)
                nc.vector.reciprocal(recips[:, 2 * j : 2 * j + 2], tot2[:])

                w2b = pool.tile([128, 2, K], bf16)
                nc.vector.tensor_copy(w2b[:], w2[:])
                offs32 = pool.tile([128, 2, K], i32)
                nc.vector.tensor_tensor(offs32[:], idx2[:], kvec[:], op=ALU.add)
                offs = pool.tile([128, 2, K], i16)
                nc.vector.tensor_copy(offs[:], offs32[:])

                eqw = pool.tile([128, 2, 512], bf16)
                nc.gpsimd.local_scatter(
                    eqw.rearrange("p t c -> p (t c)"),
                    w2b.rearrange("p t k -> p (t k)"),
                    offs.rearrange("p t k -> p (t k)"),
                    channels=128,
                    num_elems=1024,
                    num_idxs=16,
                )

                s1 = pool.tile([128, 2, 256], bf16)
                nc.vector.tensor_tensor(s1[:], eqw[:, :, 0:256], eqw[:, :, 256:512], op=ALU.add)
                s2 = pool.tile([128, 2, 128], bf16)
                nc.vector.tensor_tensor(s2[:], s1[:, :, 0:128], s1[:, :, 128:256], op=ALU.add)
                agg2 = pool.tile([128, 2, E], bf16)
                nc.vector.tensor_tensor(agg2[:], s2[:, :, 0:64], s2[:, :, 64:128], op=ALU.add)

                aggT2 = pool.tile([128, 128], bf16)
                nc.sync.dma_start_transpose(aggT2[:], agg2.rearrange("p t e -> p (t e)"))

                for t in range(2):
                    col = 2 * j + t
                    lhsT = aggT2[64 * t : 64 * t + 64, :]
                    half = slice(64 * t, 64 * t + 64)
                    pps = ppool.tile([128, D], f32)
                    nc.tensor.matmul(pps[:], lhsT, vb[half, :], start=True, stop=True)
                    m1 = pspool.tile([128, E], f32)
                    nc.tensor.matmul(m1[:], lhsT, gb[half, :], start=True, stop=True)

                    scr = pool.tile([128, E], bf16)
                    nc.vector.scalar_tensor_tensor(
                        out=scr[:],
                        in0=m1[:],
                        scalar=1.0,
                        in1=agg2[:, t, :],
                        op0=ALU.mult,
                        op1=ALU.mult,
                        accum_out=sqnorms[:, col : col + 1],
                    )

                    osb = opool.tile([128, D], bf16)
                    nc.scalar.activation(
                        osb[:], pps[:], AF.Copy, bias=0.0, scale=recips[:, col : col + 1]
                    )
                    nc.sync.dma_start(path_d[r0 + 128 * t : r0 + 128 * (t + 1), :], osb[:])

            # ---- efficiency epilogue ----
            norms = cpool.tile([128, NT], f32)
            nc.scalar.activation(norms[:], sqnorms[:], AF.Sqrt)
            nsc = cpool.tile([128, NT], f32)
            nc.vector.tensor_tensor(nsc[:], norms[:], recips[:], op=ALU.mult)
            effp = cpool.tile([128, 1], f32)
            nc.vector.tensor_reduce(effp[:], nsc[:], axis=mybir.AxisListType.X, op=ALU.add)
            nc.sync.dma_start(eff_d[:], effp[:])

    nc.compile()
    return nc


def _get_nc():
    if "nc" not in _CACHE:
        _CACHE["nc"] = _build()
    return _CACHE["nc"]


def _run(in_maps, trace=False):
    from concourse.bass_utils import run_bass_kernel_spmd

    nc = _get_nc()
    return run_bass_kernel_spmd(nc, in_maps, list(range(N_CORES)), trace=trace)


def _make_in_maps(expert_indices, expert_weights, vertices):
    import ml_dtypes

    idx = np.ascontiguousarray(np.asarray(expert_indices, dtype=np.int32))
    w = np.ascontiguousarray(np.asarray(expert_weights, dtype=np.float32))
    v = np.asarray(vertices, dtype=np.float32)
    vbf = np.ascontiguousarray(
        np.broadcast_to(v.astype(ml_dtypes.bfloat16), (2, E, D))
    )
    vtbf = np.ascontiguousarray(v.T.astype(ml_dtypes.bfloat16))
    in_maps = []
    for i in range(N_CORES):
        s = slice(i * B_CORE, (i + 1) * B_CORE)
        in_maps.append(
            {
                "expert_indices": np.ascontiguousarray(idx[s]),
                "expert_weights": np.ascontiguousarray(w[s]),
                "vertices_bf": vbf,
                "vertices_t_bf": vtbf,
            }
        )
    return in_maps


def _assemble(results):
    path = np.concatenate(
        [np.asarray(r["path"]).astype(np.float32) for r in results], axis=0
    )
    eff = sum(float(np.asarray(r["eff"], dtype=np.float64).sum()) for r in results)
    eff = np.float32(eff / B_FULL)
    return path, eff


def kernel(expert_indices, expert_weights, vertices):
    in_maps = _make_in_maps(expert_indices, expert_weights, vertices)
    res = _run(in_maps, trace=False)
    return _assemble(res.results)


# revision 13
# speedup vs baseline: 2.4696x; 2.4696x over previous
"""Trainium2 Bass kernel for nn_Amplituedro (weighted embedding lookup).

path[b] = (sum_k w[b,k] * vertices[idx[b,k]]) / sum_k w[b,k]
eff     = mean_b ||path[b]||

Data-parallel over batch: 8 cores x 8192 rows. Per core, 8 groups of
8 x 128-row tiles with an interleaved row mapping (row = g*1024 + p*8 + t)
so per-partition DRAM accesses are contiguous (256B loads / 8KB stores).

Per group:
  - one DMA load each for indices [128,8,8] i32 and weights [128,8,8] f32
  - DVE: row totals -> reciprocals; weights -> bf16; scatter offsets
    offs = 512*(t%2) + 64*k + idx (int16)
  - GPSIMD local_scatter x4: one-hot rows eqw[b, (t%2)*512 + 64*k + e] = w_bf
  - DVE add-tree x3 (8 tiles wide): reduce k -> agg8 [128, 8, 64] bf16
  - per pair: PE transpose (identity matmul) -> aggT [128,128] bf16
  - per tile: PE matmuls path = aggT_h.T @ V (N=512, psum f32) and
    m1 = aggT_h.T @ G (G = V V^T, N=64) sharing the loaded weights
  - DVE scalar_tensor_tensor: sqnorm_raw[b] = sum_e m1[b,e]*agg[b,e]
  - ACT/DVE: copy psum -> SBUF bf16 scaled by 1/total (the normalization)
  - one 1MB DMA store per group
Epilogue: norm = sqrt(sqnorm_raw) * recip; per-partition partial sums.
Host: concat path shards (bf16 -> f32), sum efficiency partials / B.
"""

import numpy as np

N_CORES = 8
B_FULL = 65536
B_CORE = B_FULL // N_CORES  # 8192
K = 8
E = 64
D = 512
GT = 8  # tiles per group
NG = B_CORE // (128 * GT)  # 8 groups
NT = B_CORE // 128  # 64 tiles

_CACHE = {}


def _build():
    import concourse.bacc as bacc
    import concourse.mybir as mybir
    import concourse.tile as tile
    from concourse import masks

    f32 = mybir.dt.float32
    bf16 = mybir.dt.bfloat16
    i32 = mybir.dt.int32
    i16 = mybir.dt.int16
    ALU = mybir.AluOpType
    AF = mybir.ActivationFunctionType

    nc = bacc.Bacc(None, target_bir_lowering=False, debug=False)

    idx_d = nc.declare_dram_parameter("expert_indices", [B_CORE, K], i32, isOutput=False)
    w_d = nc.declare_dram_parameter("expert_weights", [B_CORE, K], f32, isOutput=False)
    # vertices pre-cast to bf16 on host; stacked twice so both PE partition
    # halves hold a copy (lhsT base partition must match rhs base partition).
    v_d = nc.declare_dram_parameter("vertices_bf", [2, E, D], bf16, isOutput=False)
    vt_d = nc.declare_dram_parameter("vertices_t_bf", [D, E], bf16, isOutput=False)
    path_d = nc.declare_dram_parameter("path", [B_CORE, D], bf16, isOutput=True)
    eff_d = nc.declare_dram_parameter("eff", [128, 1], f32, isOutput=True)

    with tile.TileContext(nc) as tc:
        with (
            tc.tile_pool(name="const", bufs=1) as cpool,
            tc.tile_pool(name="work", bufs=3) as pool,
            tc.tile_pool(name="aggTp", bufs=8) as apool,
            tc.tile_pool(name="out", bufs=3) as opool,
            tc.tile_pool(name="ppair", bufs=2, space="PSUM") as ppool,
            tc.tile_pool(name="ptrans", bufs=2, space="PSUM") as tpool,
            tc.tile_pool(name="pm1", bufs=2, space="PSUM") as mpool,
        ):
            # ---- constants ----
            vb = cpool.tile([128, D], bf16)
            nc.sync.dma_start(vb[:], v_d.rearrange("two e d -> (two e) d"))
            vtb = cpool.tile([128, 4, E], bf16)
            nc.sync.dma_start(vtb[:], vt_d.rearrange("(c p) e -> p c e", p=128))

            ident = cpool.tile([128, 128], bf16)
            masks.make_identity(nc, ident[:])

            # G = V @ V.T  [64, 64], replicated into both partition halves
            g_ps = ppool.tile([E, E], f32, tag="pps")
            for c in range(4):
                nc.tensor.matmul(
                    g_ps[:], vtb[:, c, :], vtb[:, c, :], start=(c == 0), stop=(c == 3)
                )
            gb = cpool.tile([128, E], bf16)
            nc.vector.tensor_copy(gb[0:64, :], g_ps[:])
            nc.vector.tensor_copy(gb[64:128, :], g_ps[:])

            # scatter offset bias: 512*(t%2) + 64*k  over free dims (t, k)
            kvec = cpool.tile([128, GT, K], i32)
            nc.gpsimd.iota(
                kvec[:], pattern=[[0, GT // 2], [512, 2], [64, K]], base=0,
                channel_multiplier=0,
            )

            recips = cpool.tile([128, NT], f32)
            sqnorms = cpool.tile([128, NT], f32)

            for g in range(NG):
                r0 = g * 128 * GT
                rows = slice(r0, r0 + 128 * GT)
                idx8 = pool.tile([128, GT, K], i32)
                nc.sync.dma_start(idx8[:], idx_d[rows, :].rearrange("(p t) k -> p t k", p=128))
                w8 = pool.tile([128, GT, K], f32)
                nc.sync.dma_start(w8[:], w_d[rows, :].rearrange("(p t) k -> p t k", p=128))

                tot8 = pool.tile([128, GT], f32)
                nc.vector.tensor_reduce(tot8[:], w8[:], axis=mybir.AxisListType.X, op=ALU.add)
                nc.vector.reciprocal(recips[:, g * GT : (g + 1) * GT], tot8[:])

                w8b = pool.tile([128, GT, K], bf16)
                nc.vector.tensor_copy(w8b[:], w8[:])
                offs32 = pool.tile([128, GT, K], i32)
                nc.vector.tensor_tensor(offs32[:], idx8[:], kvec[:], op=ALU.add)
                offs = pool.tile([128, GT, K], i16)
                nc.vector.tensor_copy(offs[:], offs32[:])

                eqw = pool.tile([128, GT, 512], bf16)
                for u in range(GT // 2):
                    nc.gpsimd.local_scatter(
                        eqw[:, 2 * u : 2 * u + 2, :].rearrange("p t c -> p (t c)"),
                        w8b[:, 2 * u : 2 * u + 2, :].rearrange("p t k -> p (t k)"),
                        offs[:, 2 * u : 2 * u + 2, :].rearrange("p t k -> p (t k)"),
                        channels=128,
                        num_elems=1024,
                        num_idxs=16,
                    )

                s1 = pool.tile([128, GT, 256], bf16)
                nc.vector.tensor_tensor(s1[:], eqw[:, :, 0:256], eqw[:, :, 256:512], op=ALU.add)
                s2 = pool.tile([128, GT, 128], bf16)
                nc.vector.tensor_tensor(s2[:], s1[:, :, 0:128], s1[:, :, 128:256], op=ALU.add)
                agg8 = pool.tile([128, GT, E], bf16)
                nc.vector.tensor_tensor(agg8[:], s2[:, :, 0:64], s2[:, :, 64:128], op=ALU.add)

                osb = opool.tile([128, GT, D], bf16)
                for u in range(GT // 2):
                    psT = tpool.tile([128, 128], bf16)
                    nc.tensor.transpose(
                        psT[:],
                        agg8[:, 2 * u : 2 * u + 2, :].rearrange("p t e -> p (t e)"),
                        ident[:],
                    )
                    aggT = apool.tile([128, 128], bf16)
                    nc.vector.tensor_copy(aggT[:], psT[:])

                    pps = ppool.tile([128, 2, D], f32, tag="pps")
                    m1 = mpool.tile([128, 2, E], f32)
                    for h in range(2):
                        t = 2 * u + h
                        col = g * GT + t
                        half = slice(64 * h, 64 * h + 64)
                        lhsT = aggT[half, :]
                        nc.tensor.matmul(pps[:, h, :], lhsT, vb[half, :], start=True, stop=True)
                        nc.tensor.matmul(m1[:, h, :], lhsT, gb[half, :], start=True, stop=True)

                        scr = pool.tile([128, E], bf16)
                        nc.vector.scalar_tensor_tensor(
                            out=scr[:],
                            in0=m1[:, h, :],
                            scalar=1.0,
                            in1=agg8[:, t, :],
                            op0=ALU.mult,
                            op1=ALU.mult,
                            accum_out=sqnorms[:, col : col + 1],
                        )
                        if h == 0:
                            nc.scalar.activation(
                                osb[:, t, :], pps[:, h, :], AF.Copy,
                                bias=0.0, scale=recips[:, col : col + 1],
                            )
                        else:
                            nc.vector.tensor_scalar(
                                out=osb[:, t, :], in0=pps[:, h, :],
                                scalar1=recips[:, col : col + 1], scalar2=None,
                                op0=ALU.mult,
                            )

                nc.sync.dma_start(
                    path_d[rows, :].rearrange("(p t) d -> p t d", p=128), osb[:]
                )

            # ---- efficiency epilogue ----
            norms = cpool.tile([128, NT], f32)
            nc.scalar.activation(norms[:], sqnorms[:], AF.Sqrt)
            nsc = cpool.tile([128, NT], f32)
            nc.vector.tensor_tensor(nsc[:], norms[:], recips[:], op=ALU.mult)
            effp = cpool.tile([128, 1], f32)
            nc.vector.tensor_reduce(effp[:], nsc[:], axis=mybir.AxisListType.X, op=ALU.add)
            nc.sync.dma_start(eff_d[:], effp[:])

    nc.compile()
    return nc


def _get_nc():
    if "nc" not in _CACHE:
        _CACHE["nc"] = _build()
    return _CACHE["nc"]


def _run(in_maps, trace=False):
    from concourse.bass_utils import run_bass_kernel_spmd

    nc = _get_nc()
    return run_bass_kernel_spmd(nc, in_maps, list(range(N_CORES)), trace=trace)


def _make_in_maps(expert_indices, expert_weights, vertices):
    import ml_dtypes

    idx = np.ascontiguousarray(np.asarray(expert_indices, dtype=np.int32))
    w = np.ascontiguousarray(np.asarray(expert_weights, dtype=np.float32))
    v = np.asarray(vertices, dtype=np.float32)
    vbf = np.ascontiguousarray(
        np.broadcast_to(v.astype(ml_dtypes.bfloat16), (2, E, D))
    )
    vtbf = np.ascontiguousarray(v.T.astype(ml_dtypes.bfloat16))
    in_maps = []
    for i in range(N_CORES):
        s = slice(i * B_CORE, (i + 1) * B_CORE)
        in_maps.append(
            {
                "expert_indices": np.ascontiguousarray(idx[s]),
                "expert_weights": np.ascontiguousarray(w[s]),
                "vertices_bf": vbf,
                "vertices_t_bf": vtbf,
            }
        )
    return in_maps


def _assemble(results):
    path = np.concatenate(
        [np.asarray(r["path"]).astype(np.float32) for r in results], axis=0
    )
    eff = sum(float(np.asarray(r["eff"], dtype=np.float64).sum()) for r in results)
    eff = np.float32(eff / B_FULL)
    return path, eff


def kernel(expert_indices, expert_weights, vertices):
    in_maps = _make_in_maps(expert_indices, expert_weights, vertices)
    res = _run(in_maps, trace=False)
    return _assemble(res.results)


# revision 16
# speedup vs baseline: 2.8391x; 1.1496x over previous
"""Trainium2 Bass kernel for nn_Amplituedro (weighted embedding lookup).

path[b] = (sum_k w[b,k] * vertices[idx[b,k]]) / sum_k w[b,k]
eff     = mean_b ||path[b]||

Data-parallel over batch: 8 cores x 8192 rows. Per core, 8 groups of
8 x 128-row tiles with an interleaved row mapping (row = g*1024 + p*8 + t)
so per-partition DRAM accesses are contiguous (256B loads / 8KB stores).

Per group:
  - one DMA load each for indices [128,8,8] i32 and weights [128,8,8] f32
  - DVE: row totals -> reciprocals; weights -> bf16; scatter offsets
    offs = 512*(t%2) + 64*k + idx (int16)
  - GPSIMD local_scatter x4: one-hot rows eqw[b, (t%2)*512 + 64*k + e] = w_bf
  - DVE add-tree x3 (8 tiles wide): reduce k -> agg8 [128, 8, 64] bf16
  - per pair: PE transpose (identity matmul) -> aggT [128,128] bf16
  - per tile: PE matmuls path = aggT_h.T @ V (N=512, psum f32) and
    m1 = aggT_h.T @ G (G = V V^T, N=64) sharing the loaded weights
  - DVE scalar_tensor_tensor: sqnorm_raw[b] = sum_e m1[b,e]*agg[b,e]
  - ACT/DVE: copy psum -> SBUF bf16 scaled by 1/total (the normalization)
  - one 1MB DMA store per group
Epilogue: norm = sqrt(sqnorm_raw) * recip; per-partition partial sums.
Host: concat path shards (bf16 -> f32), sum efficiency partials / B.
"""

import numpy as np

N_CORES = 8
B_FULL = 65536
B_CORE = B_FULL // N_CORES  # 8192
K = 8
E = 64
D = 512
GT = 8  # tiles per group
NG = B_CORE // (128 * GT)  # 8 groups
NT = B_CORE // 128  # 64 tiles

_CACHE = {}


def _build():
    import concourse.bacc as bacc
    import concourse.mybir as mybir
    import concourse.tile as tile
    from concourse import masks

    f32 = mybir.dt.float32
    bf16 = mybir.dt.bfloat16
    i32 = mybir.dt.int32
    i16 = mybir.dt.int16
    ALU = mybir.AluOpType
    AF = mybir.ActivationFunctionType

    nc = bacc.Bacc(None, target_bir_lowering=False, debug=False)

    idx_d = nc.declare_dram_parameter("expert_indices", [B_CORE, K], i32, isOutput=False)
    w_d = nc.declare_dram_parameter("expert_weights", [B_CORE, K], f32, isOutput=False)
    # vertices pre-cast to bf16 on host; stacked twice so both PE partition
    # halves hold a copy (lhsT base partition must match rhs base partition).
    v_d = nc.declare_dram_parameter("vertices_bf", [2, E, D], bf16, isOutput=False)
    vt_d = nc.declare_dram_parameter("vertices_t_bf", [D, E], bf16, isOutput=False)
    path_d = nc.declare_dram_parameter("path", [B_CORE, D], bf16, isOutput=True)
    eff_d = nc.declare_dram_parameter("eff", [128, 1], f32, isOutput=True)

    with tile.TileContext(nc) as tc:
        with (
            tc.tile_pool(name="const", bufs=1) as cpool,
            tc.tile_pool(name="work", bufs=3) as pool,
            tc.tile_pool(name="aggTp", bufs=8) as apool,
            tc.tile_pool(name="out", bufs=3) as opool,
            tc.tile_pool(name="ppair", bufs=2, space="PSUM") as ppool,
            tc.tile_pool(name="ptrans", bufs=2, space="PSUM") as tpool,
            tc.tile_pool(name="pm1", bufs=2, space="PSUM") as mpool,
        ):
            # ---- constants ----
            vb = cpool.tile([128, D], bf16)
            nc.sync.dma_start(vb[:], v_d.rearrange("two e d -> (two e) d"))
            vtb = cpool.tile([128, 4, E], bf16)
            nc.sync.dma_start(vtb[:], vt_d.rearrange("(c p) e -> p c e", p=128))

            ident = cpool.tile([128, 128], bf16)
            masks.make_identity(nc, ident[:])

            # G = V @ V.T  [64, 64], replicated into both partition halves
            g_ps = ppool.tile([E, E], f32, tag="pps")
            for c in range(4):
                nc.tensor.matmul(
                    g_ps[:], vtb[:, c, :], vtb[:, c, :], start=(c == 0), stop=(c == 3)
                )
            gb = cpool.tile([128, E], bf16)
            nc.vector.tensor_copy(gb[0:64, :], g_ps[:])
            nc.vector.tensor_copy(gb[64:128, :], g_ps[:])

            # scatter offset bias: 512*(t%2) + 64*k  over free dims (t, k)
            kvec = cpool.tile([128, GT, K], i32)
            nc.gpsimd.iota(
                kvec[:], pattern=[[0, GT // 2], [512, 2], [64, K]], base=0,
                channel_multiplier=0,
            )

            recips = cpool.tile([128, NT], f32)
            sqnorms = cpool.tile([128, NT], f32)

            for g in range(NG):
                r0 = g * 128 * GT
                rows = slice(r0, r0 + 128 * GT)
                idx8 = pool.tile([128, GT, K], i32)
                nc.sync.dma_start(idx8[:], idx_d[rows, :].rearrange("(p t) k -> p t k", p=128))
                w8 = pool.tile([128, GT, K], f32)
                nc.sync.dma_start(w8[:], w_d[rows, :].rearrange("(p t) k -> p t k", p=128))

                tot8 = pool.tile([128, GT], f32)
                nc.vector.tensor_reduce(tot8[:], w8[:], axis=mybir.AxisListType.X, op=ALU.add)
                nc.vector.reciprocal(recips[:, g * GT : (g + 1) * GT], tot8[:])

                # normalize during the bf16 cast: w' = w / total  (per-tile scalar)
                w8b = pool.tile([128, GT, K], bf16)
                for t in range(GT):
                    col = g * GT + t
                    nc.vector.tensor_scalar(
                        out=w8b[:, t, :], in0=w8[:, t, :],
                        scalar1=recips[:, col : col + 1], scalar2=None, op0=ALU.mult,
                    )
                offs32 = pool.tile([128, GT, K], i32)
                nc.vector.tensor_tensor(offs32[:], idx8[:], kvec[:], op=ALU.add)
                offs = pool.tile([128, GT, K], i16)
                nc.vector.tensor_copy(offs[:], offs32[:])

                eqw = pool.tile([128, GT, 512], bf16)
                for u in range(GT // 2):
                    nc.gpsimd.local_scatter(
                        eqw[:, 2 * u : 2 * u + 2, :].rearrange("p t c -> p (t c)"),
                        w8b[:, 2 * u : 2 * u + 2, :].rearrange("p t k -> p (t k)"),
                        offs[:, 2 * u : 2 * u + 2, :].rearrange("p t k -> p (t k)"),
                        channels=128,
                        num_elems=1024,
                        num_idxs=16,
                    )

                s1 = pool.tile([128, GT, 256], bf16)
                nc.vector.tensor_tensor(s1[:], eqw[:, :, 0:256], eqw[:, :, 256:512], op=ALU.add)
                s2 = pool.tile([128, GT, 128], bf16)
                nc.vector.tensor_tensor(s2[:], s1[:, :, 0:128], s1[:, :, 128:256], op=ALU.add)
                agg8 = pool.tile([128, GT, E], bf16)
                nc.vector.tensor_tensor(agg8[:], s2[:, :, 0:64], s2[:, :, 64:128], op=ALU.add)

                osb = opool.tile([128, GT, D], bf16)
                for u in range(GT // 2):
                    psT = tpool.tile([128, 128], bf16)
                    nc.tensor.transpose(
                        psT[:],
                        agg8[:, 2 * u : 2 * u + 2, :].rearrange("p t e -> p (t e)"),
                        ident[:],
                    )
                    aggT = apool.tile([128, 128], bf16)
                    nc.scalar.activation(aggT[:], psT[:], AF.Copy, bias=0.0)

                    pps = ppool.tile([128, 2, D], f32, tag="pps")
                    m1 = mpool.tile([128, 2, E], f32)
                    for h in range(2):
                        t = 2 * u + h
                        col = g * GT + t
                        half = slice(64 * h, 64 * h + 64)
                        lhsT = aggT[half, :]
                        nc.tensor.matmul(pps[:, h, :], lhsT, vb[half, :], start=True, stop=True)
                        nc.tensor.matmul(m1[:, h, :], lhsT, gb[half, :], start=True, stop=True)

                        scr = pool.tile([128, E], bf16)
                        nc.vector.scalar_tensor_tensor(
                            out=scr[:],
                            in0=m1[:, h, :],
                            scalar=1.0,
                            in1=agg8[:, t, :],
                            op0=ALU.mult,
                            op1=ALU.mult,
                            accum_out=sqnorms[:, col : col + 1],
                        )
                    # agg already normalized -> plain pair-wide psum->sbuf copy
                    if u == 0:
                        nc.vector.tensor_copy(
                            osb[:, 2 * u : 2 * u + 2, :], pps[:]
                        )
                    else:
                        nc.scalar.activation(
                            osb[:, 2 * u : 2 * u + 2, :], pps[:], AF.Copy, bias=0.0
                        )

                nc.sync.dma_start(
                    path_d[rows, :].rearrange("(p t) d -> p t d", p=128), osb[:]
                )

            # ---- efficiency epilogue ----
            # agg was pre-normalized, so sqnorms[b] is already ||path_b||^2
            norms = cpool.tile([128, NT], f32)
            nc.scalar.activation(norms[:], sqnorms[:], AF.Sqrt)
            effp = cpool.tile([128, 1], f32)
            nc.vector.tensor_reduce(effp[:], norms[:], axis=mybir.AxisListType.X, op=ALU.add)
            nc.sync.dma_start(eff_d[:], effp[:])

    nc.compile()
    return nc


def _get_nc():
    if "nc" not in _CACHE:
        _CACHE["nc"] = _build()
    return _CACHE["nc"]


def _run(in_maps, trace=False):
    from concourse.bass_utils import run_bass_kernel_spmd

    nc = _get_nc()
    return run_bass_kernel_spmd(nc, in_maps, list(range(N_CORES)), trace=trace)


def _make_in_maps(expert_indices, expert_weights, vertices):
    import ml_dtypes

    idx = np.ascontiguousarray(np.asarray(expert_indices, dtype=np.int32))
    w = np.ascontiguousarray(np.asarray(expert_weights, dtype=np.float32))
    v = np.asarray(vertices, dtype=np.float32)
    vbf = np.ascontiguousarray(
        np.broadcast_to(v.astype(ml_dtypes.bfloat16), (2, E, D))
    )
    vtbf = np.ascontiguousarray(v.T.astype(ml_dtypes.bfloat16))
    in_maps = []
    for i in range(N_CORES):
        s = slice(i * B_CORE, (i + 1) * B_CORE)
        in_maps.append(
            {
                "expert_indices": np.ascontiguousarray(idx[s]),
                "expert_weights": np.ascontiguousarray(w[s]),
                "vertices_bf": vbf,
                "vertices_t_bf": vtbf,
            }
        )
    return in_maps


def _assemble(results):
    path = np.concatenate(
        [np.asarray(r["path"]).astype(np.float32) for r in results], axis=0
    )
    eff = sum(float(np.asarray(r["eff"], dtype=np.float64).sum()) for r in results)
    eff = np.float32(eff / B_FULL)
    return path, eff


def kernel(expert_indices, expert_weights, vertices):
    in_maps = _make_in_maps(expert_indices, expert_weights, vertices)
    res = _run(in_maps, trace=False)
    return _assemble(res.results)
